# revision 8
# baseline (speedup 1.0000x reference)
"""NonMaxSuppression (5x5 local max, thr=0) on 8 trn2 NeuronCores.

Input : scores [8, 1, 2048, 2048] fp32 (full).
Output: [2, 2_000_000] int32 — (h, w) coords of survivors in global
        row-major order, padded with -1 (matches jnp.nonzero(size=...)).

Data-parallel: image b -> core b. Each core computes the dense survivor
mask for its image on-device; the host unshards (nonzero + concat + pad).

Device algorithm (V-pass first, then H-pass; all fp32-exact):
  partitions = 16-row slabs (+2 halo rows each side, zero-padded);
  variable-width column strips (narrow first strip so compute starts
  early). V pass: 5-max across rows via pair pyramid (P2 row pairs,
  T3 = 4-row maxes, V5 = stt combine). H pass on V5 (16 rows only —
  this is why V goes first; halo rows hit only the cheap V stages):
  HP1 col pairs, HTT 4-col maxes, M5 = stt combine with TINY folded.
  mask = (x >= M5) exact: M5 >= x always so x >= M5 iff x == M5;
  TINY (1e-38) rejects x <= 0 (scores are randn, |x| >> 1e-38 when > 0).
  Halo zeros are DMA'd from a tiny host zeros tensor (no memset on the
  engine critical path).
"""
import sys

sys.path.insert(0, "/opt/trn_rl_repo")
import numpy as np

import concourse.bass as bass
from concourse import mybir
from concourse.bass_utils import run_bass_kernel_spmd

B, H, W = 8, 2048, 2048
NCORES = 8
MAX_KEYPOINTS = 2_000_000
TINY = 1e-38  # smallest normal fp32 territory; > 0, < any positive score

ROWS = 16            # image rows per partition (128 * 16 = 2048)
FR = ROWS + 4        # frame rows incl. 2-row halo each side
# strip widths (sum = 2048): narrow first strip so compute starts early,
# narrow last strip so the final mask store (the only unoverlapped one)
# is tiny.
WIDTHS = [192, 384, 512, 512, 352, 96]
STARTS = [0, 192, 576, 1088, 1600, 1952]
NS = len(WIDTHS)
FCMAX = 512 + 4      # frame buffer width

f32 = mybir.dt.float32
u8 = mybir.dt.uint8


def _dram_ap(t, offset, pattern):
    return bass.AP(tensor=t, offset=offset, ap=pattern)


def _build():
    nc = bass.Bass()
    x_in = nc.declare_dram_parameter("scores", [H, W], f32, isOutput=False)
    z_in = nc.declare_dram_parameter("zpad", [FR, FCMAX], f32, isOutput=False)
    m_out = nc.declare_dram_parameter("mask", [H, W], u8, isOutput=True)

    with (
        nc.sbuf_tensor("xb0", [128, FR, FCMAX], f32) as xb0,
        nc.sbuf_tensor("xb1", [128, FR, FCMAX], f32) as xb1,
        # scr rows 0:10 = P2, rows 10:19 = T3; M5 aliases rows 0:16 (the
        # P2/T3 values are dead by the time the H pass writes M5 — the DVE
        # runs each strip's 9 ops serially).
        nc.sbuf_tensor("scr", [128, FR, FCMAX], f32) as scr,
        nc.sbuf_tensor("v5", [128, ROWS, FCMAX], f32) as v5,
        nc.sbuf_tensor("hp1", [128, ROWS, FCMAX // 2], f32) as hp1,
        nc.sbuf_tensor("htt", [128, ROWS, FCMAX // 2], f32) as htt,
        nc.sbuf_tensor("msk", [128, ROWS, 512], u8) as msk,
        nc.Block() as block,
        nc.semaphore("load_sem") as load_sem,
        nc.semaphore("free_sem") as free_sem,
        nc.semaphore("out_sem") as out_sem,
    ):
        bufs = [xb0, xb1]

        def strip_geom(s):
            start, w = STARTS[s], WIDTHS[s]
            c0 = max(0, start - 2)
            c1 = min(W, start + w + 2)
            dc = c0 - (start - 2)  # dst col offset inside the frame
            return start, w, c0, dc, c1 - c0

        # DMAs per strip load-group (3 main loads per strip; the 5 one-time
        # halo zero fills are issued from the scalar engine in parallel and
        # counted in strip 0's group; the last strip has a right-halo
        # refill). Cumulative sem thresholds, 16 per DMA.
        ndma = [3] * NS
        ndma[0] += 5
        ndma[NS - 1] += 1
        cum = [16 * sum(ndma[: s + 1]) for s in range(NS)]

        @block.sync
        def _(sync):
            for s in range(NS):
                xb = bufs[s % 2]
                start, w, c0, dc, cw = strip_geom(s)
                if s >= 2:
                    sync.wait_ge(free_sem, s - 1)
                if s >= 1:
                    # serialize strip loads: earlier strips get the full DMA
                    # bandwidth, so compute never waits on a load that's
                    # sharing bandwidth with a later prefetch
                    sync.wait_ge(load_sem, cum[s - 1])
                # partitions 1..126: image rows 16p-2 .. 16p+17
                sync.dma_start(
                    out=xb[1:127, :, dc : dc + cw],
                    in_=_dram_ap(
                        x_in, 14 * W + c0, [[16 * W, 126], [W, FR], [1, cw]]
                    ),
                ).then_inc(load_sem, 16)
                # partition 0: image rows 0..17 -> frame rows 2..19
                sync.dma_start(
                    out=xb[0:1, 2:FR, dc : dc + cw],
                    in_=_dram_ap(x_in, c0, [[0, 1], [W, FR - 2], [1, cw]]),
                ).then_inc(load_sem, 16)
                # partition 127: image rows 2030..2047 -> frame rows 0..17
                sync.dma_start(
                    out=xb[127:128, 0 : FR - 2, dc : dc + cw],
                    in_=_dram_ap(
                        x_in, 2030 * W + c0, [[0, 1], [W, FR - 2], [1, cw]]
                    ),
                ).then_inc(load_sem, 16)
                if s == NS - 1:
                    # right col halo: frame cols cw..cw+2 (beyond image)
                    sync.dma_start(
                        out=xb[:, :, dc + cw : dc + cw + 2],
                        in_=_dram_ap(z_in, 0, [[0, 128], [FCMAX, FR], [1, 2]]),
                    ).then_inc(load_sem, 16)

        @block.vector
        def _(v):
            A = mybir.AluOpType
            p2 = scr[:, 0:10, :]
            t3 = scr[:, 10:19, :]
            for s in range(NS):
                xb = bufs[s % 2]
                start, w, c0, dc, cw = strip_geom(s)
                fw = w + 4
                hw = w // 2
                v.wait_ge(load_sem, cum[s])
                # --- V pass (on fw cols incl. col halo) ---
                v.tensor_tensor(
                    out=p2[:, :, 0:fw], in0=xb[:, 0:FR:2, 0:fw],
                    in1=xb[:, 1:FR:2, 0:fw], op=A.max,
                )
                v.tensor_tensor(
                    out=t3[:, 0:9, 0:fw], in0=p2[:, 0:9, 0:fw],
                    in1=p2[:, 1:10, 0:fw], op=A.max,
                )
                # v5[2j] = max(t3[j], xf[2j+4]); v5[2j+1] = max(t3[j+1], xf[2j+1])
                v.tensor_tensor(
                    out=v5[:, 0:ROWS:2, 0:fw], in0=t3[:, 0:8, 0:fw],
                    in1=xb[:, 4:FR:2, 0:fw], op=A.max,
                )
                v.tensor_tensor(
                    out=v5[:, 1:ROWS:2, 0:fw], in0=t3[:, 1:9, 0:fw],
                    in1=xb[:, 1 : ROWS + 1 : 2, 0:fw], op=A.max,
                )
                # --- H pass (16 rows) ---
                v.tensor_tensor(
                    out=hp1[:, :, 0 : fw // 2], in0=v5[:, :, 0:fw:2],
                    in1=v5[:, :, 1:fw:2], op=A.max,
                )
                v.tensor_tensor(
                    out=htt[:, :, 0 : fw // 2 - 1], in0=hp1[:, :, 0 : fw // 2 - 1],
                    in1=hp1[:, :, 1 : fw // 2], op=A.max,
                )
                # m5[2m] = max(htt[m], v5[2m+4], TINY)
                m5 = scr[:, 0:ROWS, 0:w]
                v.scalar_tensor_tensor(
                    out=m5[:, :, 0:w:2], in0=htt[:, :, 0:hw], scalar=TINY,
                    in1=v5[:, :, 4 : 4 + w : 2], op0=A.max, op1=A.max,
                )
                # m5[2m+1] = max(htt[m+1], v5[2m+1], TINY)
                v.scalar_tensor_tensor(
                    out=m5[:, :, 1:w:2], in0=htt[:, :, 1 : hw + 1], scalar=TINY,
                    in1=v5[:, :, 1 : 1 + w : 2], op0=A.max, op1=A.max,
                )
                # --- mask --- (msk is reused each strip; wait for the
                # previous strip's store to drain before overwriting)
                if s >= 1:
                    v.wait_ge(out_sem, 16 * s)
                v.tensor_tensor(
                    out=msk[:, :, 0:w], in0=xb[:, 2 : 2 + ROWS, 2 : 2 + w],
                    in1=m5[:, :, 0:w], op=A.is_ge,
                )
                v.drain().then_inc(free_sem, 1)

        @block.scalar
        def _(sc):
            # one-time halo zero fills, issued here so they don't serialize
            # behind the strip-0 main loads on the sync engine
            sc.dma_start(
                out=xb0[:, :, 0:2],
                in_=_dram_ap(z_in, 0, [[0, 128], [FCMAX, FR], [1, 2]]),
            ).then_inc(load_sem, 16)
            for xbz in bufs:
                # partition 0: top halo rows (image rows -2, -1)
                sc.dma_start(
                    out=xbz[0:1, 0:2, :],
                    in_=_dram_ap(z_in, 0, [[0, 1], [FCMAX, 2], [1, FCMAX]]),
                ).then_inc(load_sem, 16)
                # partition 127: bottom halo rows (image rows 2048, 2049)
                sc.dma_start(
                    out=xbz[127:128, FR - 2 : FR, :],
                    in_=_dram_ap(z_in, 0, [[0, 1], [FCMAX, 2], [1, FCMAX]]),
                ).then_inc(load_sem, 16)
            # stream mask strips out as compute finishes them
            for s in range(NS):
                start, w = STARTS[s], WIDTHS[s]
                sc.wait_ge(free_sem, s + 1)
                sc.dma_start(
                    out=_dram_ap(
                        m_out, start, [[16 * W, 128], [W, ROWS], [1, w]]
                    ),
                    in_=msk[:, :, 0:w],
                ).then_inc(out_sem, 16)
            sc.wait_ge(out_sem, 16 * NS)

    return nc


_nc = None
_ZPAD = np.zeros((FR, FCMAX), dtype=np.float32)


def kernel(scores: np.ndarray) -> np.ndarray:
    global _nc
    scores = np.ascontiguousarray(np.asarray(scores), dtype=np.float32)
    assert scores.shape == (B, 1, H, W), scores.shape
    if _nc is None:
        _nc = _build()
    in_maps = [
        {"scores": np.ascontiguousarray(scores[b, 0]), "zpad": _ZPAD}
        for b in range(NCORES)
    ]
    res = run_bass_kernel_spmd(_nc, in_maps, list(range(NCORES)), trace=False)
    hs, ws = [], []
    for b in range(NCORES):
        mask = res.results[b]["mask"]
        idx = np.flatnonzero(mask)  # row-major == (h, w) lexicographic
        hs.append((idx // W).astype(np.int32))
        ws.append((idx % W).astype(np.int32))
    hh = np.concatenate(hs)
    ww = np.concatenate(ws)
    n = min(len(hh), MAX_KEYPOINTS)
    out = np.full((2, MAX_KEYPOINTS), -1, dtype=np.int32)
    out[0, :n] = hh[:n]
    out[1, :n] = ww[:n]
    return out


if __name__ == "__main__":
    rng = np.random.default_rng(0)
    x = rng.standard_normal((B, 1, H, W), dtype=np.float32)
    out = kernel(scores=x)
    print("out", out.shape, out.dtype, "nvalid:", int((out[0] >= 0).sum()))


# revision 15
# speedup vs baseline: 1.2112x; 1.2112x over previous
"""NonMaxSuppression (5x5 local max, thr=0) on 8 trn2 NeuronCores.

Input : scores [8, 1, 2048, 2048] fp32 (full).
Output: [2, 2_000_000] int32 — (h, w) coords of survivors in global
        row-major order, padded with -1 (matches jnp.nonzero(size=...)).

Data-parallel: image b -> core b. Each core computes the dense survivor
mask for its image on-device; the host unshards (nonzero + concat + pad).

Device algorithm (V-pass first, then H-pass; all fp32-exact):
  partitions = 16-row slabs (+2 halo rows each side, zero-padded);
  variable-width column strips (narrow first strip so compute starts
  early). V pass: 5-max across rows via pair pyramid (P2 row pairs,
  T3 = 4-row maxes, V5 = stt combine). H pass on V5 (16 rows only —
  this is why V goes first; halo rows hit only the cheap V stages):
  HP1 col pairs, HTT 4-col maxes, M5 = stt combine with TINY folded.
  mask = (x >= M5) exact: M5 >= x always so x >= M5 iff x == M5;
  TINY (1e-38) rejects x <= 0 (scores are randn, |x| >> 1e-38 when > 0).
  Halo zeros are DMA'd from a tiny host zeros tensor (no memset on the
  engine critical path).
"""
import sys

sys.path.insert(0, "/opt/trn_rl_repo")
import numpy as np

import concourse.bass as bass
from concourse import mybir
from concourse.bass_utils import run_bass_kernel_spmd

B, H, W = 8, 2048, 2048
NCORES = 8
MAX_KEYPOINTS = 2_000_000
TINY = 1e-38  # smallest normal fp32 territory; > 0, < any positive score

ROWS = 16            # image rows per partition (128 * 16 = 2048)
FR = ROWS + 4        # frame rows incl. 2-row halo each side
# strip widths (sum = 2048): narrow first strip so compute starts early,
# narrow last strip so the final mask store (the only unoverlapped one)
# is tiny.
WIDTHS = [192, 512, 512, 512, 224, 96]
STARTS = [0, 192, 704, 1216, 1728, 1952]
NS = len(WIDTHS)
FCMAX = 512 + 4      # frame buffer width

f32 = mybir.dt.float32
u8 = mybir.dt.uint8


def _dram_ap(t, offset, pattern):
    return bass.AP(tensor=t, offset=offset, ap=pattern)


FC0 = WIDTHS[0] + 4  # prestaged strip-0 frame width
FC1 = WIDTHS[1] + 4  # prestaged strip-1 frame width


def _build():
    nc = bass.Bass()
    x_in = nc.declare_dram_parameter("scores", [H, W], f32, isOutput=False)
    z_in = nc.declare_dram_parameter("zpad", [FR, FCMAX], f32, isOutput=False)
    # strips 0/1 prestaged host-side in slab layout (halos baked in) so
    # their loads are one dense descriptor per partition — HWDGE
    # descriptor generation for the strided slab loads costs 3-7us, which
    # only matters for the first loads (later ones hide under compute)
    s0_in = nc.declare_dram_parameter("strip0", [128, FR, FC0], f32, isOutput=False)
    s1_in = nc.declare_dram_parameter("strip1", [128, FR, FC1], f32, isOutput=False)
    m_out = nc.declare_dram_parameter("mask", [H, W], u8, isOutput=True)

    with (
        nc.sbuf_tensor("xb0", [128, FR, FCMAX], f32) as xb0,
        nc.sbuf_tensor("xb1", [128, FR, FCMAX], f32) as xb1,
        # scr rows 0:10 = P2, rows 10:19 = T3; M5 aliases rows 0:16 (the
        # P2/T3 values are dead by the time the H pass writes M5 — the DVE
        # runs each strip's 9 ops serially).
        nc.sbuf_tensor("scr", [128, FR, FCMAX], f32) as scr,
        nc.sbuf_tensor("v5", [128, ROWS, FCMAX], f32) as v5,
        nc.sbuf_tensor("hp1", [128, ROWS, FCMAX // 2], f32) as hp1,
        nc.sbuf_tensor("htt", [128, ROWS, FCMAX // 2], f32) as htt,
        nc.sbuf_tensor("msk", [128, ROWS, 512], u8) as msk,
        nc.Block() as block,
        nc.semaphore("load_sem") as load_sem,
        nc.semaphore("free_sem") as free_sem,
        nc.semaphore("out_sem") as out_sem,
    ):
        bufs = [xb0, xb1]

        def strip_geom(s):
            start, w = STARTS[s], WIDTHS[s]
            c0 = max(0, start - 2)
            c1 = min(W, start + w + 2)
            dc = c0 - (start - 2)  # dst col offset inside the frame
            return start, w, c0, dc, c1 - c0

        # DMAs per strip load-group; cumulative sem thresholds, 16 per DMA.
        ndma = [3] * NS
        ndma[0] = 1 + 2  # prestaged load + xb0 p0/p127 row-halo zeros
        ndma[1] = 1      # prestaged (full frame width, halos included)
        ndma[NS - 1] += 1  # right col halo refill
        cum = [16 * sum(ndma[: s + 1]) for s in range(NS)]

        @block.sync
        def _(sync):
            for s in range(NS):
                xb = bufs[s % 2]
                start, w, c0, dc, cw = strip_geom(s)
                if s >= 2:
                    sync.wait_ge(free_sem, s - 1)
                if s >= 1:
                    # serialize strip loads: earlier strips get the full DMA
                    # bandwidth, so compute never waits on a load that's
                    # sharing bandwidth with a later prefetch
                    sync.wait_ge(load_sem, cum[s - 1])
                if s == 0:
                    sync.dma_start(
                        out=xb[:, :, 0:FC0],
                        in_=_dram_ap(
                            s0_in, 0, [[FR * FC0, 128], [FC0, FR], [1, FC0]]
                        ),
                    ).then_inc(load_sem, 16)
                    continue
                if s == 1:
                    sync.dma_start(
                        out=xb[:, :, 0:FC1],
                        in_=_dram_ap(
                            s1_in, 0, [[FR * FC1, 128], [FC1, FR], [1, FC1]]
                        ),
                    ).then_inc(load_sem, 16)
                    continue
                # partitions 1..126: image rows 16p-2 .. 16p+17
                sync.dma_start(
                    out=xb[1:127, :, dc : dc + cw],
                    in_=_dram_ap(
                        x_in, 14 * W + c0, [[16 * W, 126], [W, FR], [1, cw]]
                    ),
                ).then_inc(load_sem, 16)
                # partition 0: image rows 0..17 -> frame rows 2..19
                sync.dma_start(
                    out=xb[0:1, 2:FR, dc : dc + cw],
                    in_=_dram_ap(x_in, c0, [[0, 1], [W, FR - 2], [1, cw]]),
                ).then_inc(load_sem, 16)
                # partition 127: image rows 2030..2047 -> frame rows 0..17
                sync.dma_start(
                    out=xb[127:128, 0 : FR - 2, dc : dc + cw],
                    in_=_dram_ap(
                        x_in, 2030 * W + c0, [[0, 1], [W, FR - 2], [1, cw]]
                    ),
                ).then_inc(load_sem, 16)
                if s == NS - 1:
                    # right col halo: frame cols cw..cw+2 (beyond image)
                    sync.dma_start(
                        out=xb[:, :, dc + cw : dc + cw + 2],
                        in_=_dram_ap(z_in, 0, [[0, 128], [FCMAX, FR], [1, 2]]),
                    ).then_inc(load_sem, 16)

        @block.vector
        def _(v):
            A = mybir.AluOpType
            p2 = scr[:, 0:10, :]
            t3 = scr[:, 10:19, :]
            for s in range(NS):
                xb = bufs[s % 2]
                start, w, c0, dc, cw = strip_geom(s)
                fw = w + 4
                hw = w // 2
                v.wait_ge(load_sem, cum[s])
                # --- V pass (on fw cols incl. col halo) ---
                v.tensor_tensor(
                    out=p2[:, :, 0:fw], in0=xb[:, 0:FR:2, 0:fw],
                    in1=xb[:, 1:FR:2, 0:fw], op=A.max,
                )
                v.tensor_tensor(
                    out=t3[:, 0:9, 0:fw], in0=p2[:, 0:9, 0:fw],
                    in1=p2[:, 1:10, 0:fw], op=A.max,
                )
                # v5[2j] = max(t3[j], xf[2j+4]); v5[2j+1] = max(t3[j+1], xf[2j+1])
                v.tensor_tensor(
                    out=v5[:, 0:ROWS:2, 0:fw], in0=t3[:, 0:8, 0:fw],
                    in1=xb[:, 4:FR:2, 0:fw], op=A.max,
                )
                v.tensor_tensor(
                    out=v5[:, 1:ROWS:2, 0:fw], in0=t3[:, 1:9, 0:fw],
                    in1=xb[:, 1 : ROWS + 1 : 2, 0:fw], op=A.max,
                )
                # --- H pass (16 rows) ---
                v.tensor_tensor(
                    out=hp1[:, :, 0 : fw // 2], in0=v5[:, :, 0:fw:2],
                    in1=v5[:, :, 1:fw:2], op=A.max,
                )
                v.tensor_tensor(
                    out=htt[:, :, 0 : fw // 2 - 1], in0=hp1[:, :, 0 : fw // 2 - 1],
                    in1=hp1[:, :, 1 : fw // 2], op=A.max,
                )
                # m5[2m] = max(htt[m], v5[2m+4], TINY)
                m5 = scr[:, 0:ROWS, 0:w]
                v.scalar_tensor_tensor(
                    out=m5[:, :, 0:w:2], in0=htt[:, :, 0:hw], scalar=TINY,
                    in1=v5[:, :, 4 : 4 + w : 2], op0=A.max, op1=A.max,
                )
                # m5[2m+1] = max(htt[m+1], v5[2m+1], TINY)
                v.scalar_tensor_tensor(
                    out=m5[:, :, 1:w:2], in0=htt[:, :, 1 : hw + 1], scalar=TINY,
                    in1=v5[:, :, 1 : 1 + w : 2], op0=A.max, op1=A.max,
                )
                # --- mask --- (msk is reused each strip; wait for the
                # previous strip's store to drain before overwriting)
                if s >= 1:
                    v.wait_ge(out_sem, 16 * s)
                v.tensor_tensor(
                    out=msk[:, :, 0:w], in0=xb[:, 2 : 2 + ROWS, 2 : 2 + w],
                    in1=m5[:, :, 0:w], op=A.is_ge,
                )
                v.drain().then_inc(free_sem, 1)

        @block.scalar
        def _(sc):
            # one-time row-halo zeros for xb0 beyond the prestaged strip-0
            # frame width (strip 0's prestage covers only cols 0:FC0; later
            # strips on xb0 need zeroed p0/p127 halo rows across the full
            # frame). Overlap with the prestaged load is zeros-on-zeros.
            sc.dma_start(
                out=xb0[0:1, 0:2, :],
                in_=_dram_ap(z_in, 0, [[0, 1], [FCMAX, 2], [1, FCMAX]]),
            ).then_inc(load_sem, 16)
            sc.dma_start(
                out=xb0[127:128, FR - 2 : FR, :],
                in_=_dram_ap(z_in, 0, [[0, 1], [FCMAX, 2], [1, FCMAX]]),
            ).then_inc(load_sem, 16)
            # stream mask strips out as compute finishes them
            for s in range(NS):
                start, w = STARTS[s], WIDTHS[s]
                sc.wait_ge(free_sem, s + 1)
                sc.dma_start(
                    out=_dram_ap(
                        m_out, start, [[16 * W, 128], [W, ROWS], [1, w]]
                    ),
                    in_=msk[:, :, 0:w],
                ).then_inc(out_sem, 16)
            sc.wait_ge(out_sem, 16 * NS)

    return nc


_nc = None
_ZPAD = np.zeros((FR, FCMAX), dtype=np.float32)
_SLAB_IDX = (16 * np.arange(128))[:, None] + np.arange(FR)[None, :]


def _prestage(img: np.ndarray, start: int, w: int) -> np.ndarray:
    """Slab-layout frame [128, FR, w+4] for a strip, halos baked in."""
    fc = w + 4
    pad = np.zeros((H + 4, fc), dtype=np.float32)
    c0 = max(0, start - 2)
    c1 = min(W, start + w + 2)
    dc = c0 - (start - 2)
    pad[2 : H + 2, dc : dc + (c1 - c0)] = img[:, c0:c1]
    return np.ascontiguousarray(pad[_SLAB_IDX])


def kernel(scores: np.ndarray) -> np.ndarray:
    global _nc
    scores = np.ascontiguousarray(np.asarray(scores), dtype=np.float32)
    assert scores.shape == (B, 1, H, W), scores.shape
    if _nc is None:
        _nc = _build()
    in_maps = [
        {
            "scores": np.ascontiguousarray(scores[b, 0]),
            "zpad": _ZPAD,
            "strip0": _prestage(scores[b, 0], STARTS[0], WIDTHS[0]),
            "strip1": _prestage(scores[b, 0], STARTS[1], WIDTHS[1]),
        }
        for b in range(NCORES)
    ]
    res = run_bass_kernel_spmd(_nc, in_maps, list(range(NCORES)), trace=False)
    hs, ws = [], []
    for b in range(NCORES):
        mask = res.results[b]["mask"]
        idx = np.flatnonzero(mask)  # row-major == (h, w) lexicographic
        hs.append((idx // W).astype(np.int32))
        ws.append((idx % W).astype(np.int32))
    hh = np.concatenate(hs)
    ww = np.concatenate(ws)
    n = min(len(hh), MAX_KEYPOINTS)
    out = np.full((2, MAX_KEYPOINTS), -1, dtype=np.int32)
    out[0, :n] = hh[:n]
    out[1, :n] = ww[:n]
    return out


if __name__ == "__main__":
    rng = np.random.default_rng(0)
    x = rng.standard_normal((B, 1, H, W), dtype=np.float32)
    out = kernel(scores=x)
    print("out", out.shape, out.dtype, "nvalid:", int((out[0] >= 0).sum()))
